# revision 7
# baseline (speedup 1.0000x reference)
"""Trainium2 Bass kernel for 16-head attention + proj + residual + BatchNorm.

Shapes (hardcoded): B=4, L=1024, D_MODEL=1024, N_HEAD=16, D_K=D_V=64.
Sharding: 8 cores = (batch b, query-half). Each core computes all 16 heads
for its 512 query tokens; K/V projections are recomputed per query-half
(cheaper than a cross-core reduction of the row-parallel proj matmul).
Only collective: an 8KB AllReduce of BatchNorm statistics.

Layout strategy: all host-side transposes (q/k/v passed d-major, proj_w
passed pre-transposed, masks passed in both orientations) so the device
never runs PE transposes. Scores are computed in both orientations on PE
(s for the attns output + softmax row-sums via ACT accum_out; sT for the
attn@V matmul). The boolean mask is folded into the score PSUM via a bf16
identity-matmul accumulation of (-1e6 * mask), making exp() produce exact
zeros at masked positions.
"""

import sys

if "/opt/trn_rl_repo" not in sys.path:
    sys.path.insert(0, "/opt/trn_rl_repo")

import numpy as np

N_HEAD, D, DK = 16, 1024, 64
B, L = 4, 1024
NCORES = 8
QSH = 512          # query tokens per core
INV_TEMPER = 1.0 / 32.0   # 1/sqrt(D_MODEL)
MASK_NEG = -1.0e6
BN_EPS = 1e-5
NTOK = B * L       # 4096 tokens for batch-norm stats

_CACHE = {}


def _build_program():
    import concourse.bass as bass
    import concourse.tile as tile
    from concourse import bacc, mybir
    from contextlib import ExitStack

    f32 = mybir.dt.float32
    f32r = mybir.dt.float32r
    bf16 = mybir.dt.bfloat16
    u8 = mybir.dt.uint8
    AF = mybir.ActivationFunctionType
    ALU = mybir.AluOpType

    nc = bacc.Bacc("TRN2", target_bir_lowering=False, debug=False,
                   num_devices=NCORES)

    # ---------------- DRAM I/O ----------------
    qT_d = nc.dram_tensor("qT", [D, QSH], f32r, kind="ExternalInput")
    kT_d = nc.dram_tensor("kT", [D, L], f32r, kind="ExternalInput")
    vT_d = nc.dram_tensor("vT", [D, L], f32r, kind="ExternalInput")
    msk_d = nc.dram_tensor("msk", [QSH, L], u8, kind="ExternalInput")
    mskT_d = nc.dram_tensor("mskT", [L, QSH], u8, kind="ExternalInput")
    wq_d = nc.dram_tensor("wq", [8, D, 128], f32r, kind="ExternalInput")
    wk_d = nc.dram_tensor("wk", [8, D, 128], f32r, kind="ExternalInput")
    wv_d = nc.dram_tensor("wv", [D, N_HEAD * DK], f32r, kind="ExternalInput")
    pwT_d = nc.dram_tensor("pwT", [D, D], f32r, kind="ExternalInput")
    gb_d = nc.dram_tensor("gb", [128, 16], f32, kind="ExternalInput")
    idb_d = nc.dram_tensor("idb", [128, 128], bf16, kind="ExternalInput")
    ones_d = nc.dram_tensor("ones", [128, 1], f32r, kind="ExternalInput")

    attns_o = nc.dram_tensor("attns_o", [N_HEAD, QSH, L], f32,
                             kind="ExternalOutput")
    xT_o = nc.dram_tensor("xT_o", [D, QSH], f32, kind="ExternalOutput")

    st_in_d = nc.dram_tensor("st_in", [128, 16], f32)
    st_out_d = nc.dram_tensor("st_out", [128, 16], f32, addr_space="Shared")

    with tile.TileContext(nc) as tc, ExitStack() as glob:
        pool_g = glob.enter_context(tc.tile_pool(name="glob", bufs=1))
        ps_big = glob.enter_context(
            tc.tile_pool(name="psbig", bufs=2, space="PSUM"))
        ps_sm = glob.enter_context(
            tc.tile_pool(name="pssm", bufs=2, space="PSUM"))
        ps_oh = glob.enter_context(
            tc.tile_pool(name="psoh", bufs=2, space="PSUM"))

        # ---- global SBUF residents
        idb_sb = pool_g.tile([128, 128], bf16)
        nc.sync.dma_start(idb_sb[:], idb_d[:])
        ones_sb = pool_g.tile([128, 1], f32r)
        nc.sync.dma_start(ones_sb[:], ones_d[:])
        gb_sb = pool_g.tile([128, 16], f32)
        nc.sync.dma_start(gb_sb[:], gb_d[:])

        qhT = pool_g.tile([128, 8 * 512], f32r)    # [2*64dk, hp*512 + q]
        khT = pool_g.tile([128, 8 * 1024], f32r)   # [2*64dk, hp*1024 + k]
        vh = pool_g.tile([128, 8 * 1024], f32r)    # [k%128, kt*1024 + h*64+dv]
        ocT = pool_g.tile([128, 8 * 512], f32r)    # [f%128, fc*512 + q]

        # =============== PHASE 1: projections ===============
        with ExitStack() as ph1:
            p_kv = ph1.enter_context(tc.tile_pool(name="kvT", bufs=9))
            p_qt = ph1.enter_context(tc.tile_pool(name="qTs", bufs=8))
            p_w = ph1.enter_context(tc.tile_pool(name="wstr", bufs=4))
            p_wv = ph1.enter_context(tc.tile_pool(name="wvs", bufs=8))

            qT_sb = []
            for c in range(8):
                t = p_qt.tile([128, QSH], f32r, name=f"qT{c}", tag="qT")
                nc.sync.dma_start(t[:], qT_d[c * 128:(c + 1) * 128, :])
                qT_sb.append(t)
            kT_sb = []
            for c in range(8):
                t = p_kv.tile([128, L], f32r, name=f"kT{c}", tag="kvT")
                nc.sync.dma_start(t[:], kT_d[c * 128:(c + 1) * 128, :])
                kT_sb.append(t)

            # Q / K per-head-pair projections
            for hp in range(8):
                wq_t = p_w.tile([128, 8, 128], f32r, name=f"wq{hp}", tag="w")
                nc.sync.dma_start(
                    wq_t[:], wq_d[hp].rearrange("(c p) m -> p c m", p=128))
                psq = ps_sm.tile([128, 512], f32, name=f"psq{hp}", tag="sm")
                for c in range(8):
                    nc.tensor.matmul(psq[:], wq_t[:, c, :], qT_sb[c][:],
                                     start=(c == 0), stop=(c == 7))
                nc.scalar.copy(qhT[:, hp * 512:(hp + 1) * 512], psq[:])

                wk_t = p_w.tile([128, 8, 128], f32r, name=f"wk{hp}", tag="w")
                nc.sync.dma_start(
                    wk_t[:], wk_d[hp].rearrange("(c p) m -> p c m", p=128))
                psk = ps_big.tile([128, 1024], f32, name=f"psk{hp}", tag="big")
                for half in range(2):
                    sl = slice(half * 512, (half + 1) * 512)
                    for c in range(8):
                        nc.tensor.matmul(psk[:, sl], wk_t[:, c, :],
                                         kT_sb[c][:, sl],
                                         start=(c == 0), stop=(c == 7))
                nc.scalar.copy(khT[:, hp * 1024:(hp + 1) * 1024], psk[:])

            # V projection: vh[kt] = vT[:, kt-chunk].T @ wv  -> (128, 16*64)
            wv_sb = []
            for c in range(8):
                t = p_wv.tile([128, 1024], f32r, name=f"wv{c}", tag="wv")
                nc.sync.dma_start(t[:], wv_d[c * 128:(c + 1) * 128, :])
                wv_sb.append(t)
            vT_sb = []
            for c in range(8):
                t = p_kv.tile([128, L], f32r, name=f"vT{c}", tag="kvT")
                nc.sync.dma_start(t[:], vT_d[c * 128:(c + 1) * 128, :])
                vT_sb.append(t)
            for kt in range(8):
                psv = ps_big.tile([128, 1024], f32, name=f"psv{kt}", tag="big")
                for half in range(2):
                    sl = slice(half * 512, (half + 1) * 512)
                    for c in range(8):
                        nc.tensor.matmul(
                            psv[:, sl],
                            vT_sb[c][:, kt * 128:(kt + 1) * 128],
                            wv_sb[c][:, sl],
                            start=(c == 0), stop=(c == 7))
                nc.scalar.copy(vh[:, kt * 1024:(kt + 1) * 1024], psv[:])

        # =============== PHASE 2: attention ===============
        with ExitStack() as ph2:
            p_mu = ph2.enter_context(tc.tile_pool(name="mu8", bufs=1))
            p_mb = ph2.enter_context(tc.tile_pool(name="mbias", bufs=1))
            p_e = ph2.enter_context(tc.tile_pool(name="epool", bufs=4))
            p_et = ph2.enter_context(tc.tile_pool(name="etpool", bufs=4))
            p_r = ph2.enter_context(tc.tile_pool(name="rpool", bufs=1))
            p_rt = ph2.enter_context(tc.tile_pool(name="rtpool", bufs=2))
            p_rb = ph2.enter_context(tc.tile_pool(name="rbpool", bufs=2))
            p_pw = ph2.enter_context(tc.tile_pool(name="pwpool", bufs=8))
            p_x = ph2.enter_context(tc.tile_pool(name="xpool", bufs=9))
            p_st = ph2.enter_context(tc.tile_pool(name="stpool", bufs=1))

            # masks -> additive bias (bf16)
            mu_t = p_mu.tile([128, 4, 1024], u8, name="mu", tag="mu")
            nc.sync.dma_start(mu_t[:], msk_d[:].rearrange("(t p) k -> p t k",
                                                          p=128))
            mb = p_mb.tile([128, 4, 1024], bf16, name="mb", tag="mb")
            nc.vector.tensor_scalar(mb[:], mu_t[:], MASK_NEG, None, ALU.mult)
            muT_t = p_mu.tile([128, 8, 512], u8, name="muT", tag="mu")
            nc.sync.dma_start(muT_t[:], mskT_d[:].rearrange("(t p) k -> p t k",
                                                            p=128))
            mbT = p_mb.tile([128, 8, 512], bf16, name="mbT", tag="mbT")
            nc.vector.tensor_scalar(mbT[:], muT_t[:], MASK_NEG, None, ALU.mult)

            # proj weights loaded here (SBUF freed by phase-1 exit)
            pwT_sb = []
            for c in range(8):
                t = p_pw.tile([128, 1024], f32r, name=f"pwT{c}", tag="pw")
                nc.sync.dma_start(t[:], pwT_d[c * 128:(c + 1) * 128, :])
                pwT_sb.append(t)

            r_all = p_r.tile([128, 64], f32, name="r_all", tag="r")
            ri_all = p_r.tile([128, 64], f32, name="ri_all", tag="ri")

            for h in range(N_HEAD):
                hp, hb = h // 2, h % 2
                base = hb * 64
                qh_sl = qhT[base:base + 64, hp * 512:(hp + 1) * 512]

                # ---- e-path: s = qh @ khT + maskbias; softmax numerator
                e_tiles = []
                for qt in range(4):
                    pse = ps_big.tile([128, 1024], f32, name=f"pse{h}_{qt}",
                                      tag="big")
                    for half in range(2):
                        sl = slice(half * 512, (half + 1) * 512)
                        nc.tensor.matmul(pse[:, sl], idb_sb[:],
                                         mb[:, qt, sl],
                                         start=True, stop=False)
                        nc.tensor.matmul(
                            pse[:, sl],
                            qhT[base:base + 64,
                                hp * 512 + qt * 128:hp * 512 + (qt + 1) * 128],
                            khT[base:base + 64, hp * 1024 + half * 512:
                                hp * 1024 + (half + 1) * 512],
                            start=False, stop=True)
                    e_t = p_e.tile([128, 1024], f32, name=f"e{h}_{qt}",
                                   tag="e")
                    nc.scalar.activation(e_t[:], pse[:], AF.Exp,
                                         scale=INV_TEMPER,
                                         accum_out=r_all[:, h * 4 + qt:h * 4 + qt + 1])
                    e_tiles.append(e_t)
                nc.vector.reciprocal(ri_all[:, h * 4:h * 4 + 4],
                                     r_all[:, h * 4:h * 4 + 4])
                for qt in range(4):
                    nc.vector.tensor_scalar(e_tiles[qt][:], e_tiles[qt][:],
                                            ri_all[:, h * 4 + qt:h * 4 + qt + 1],
                                            None, ALU.mult)
                    nc.sync.dma_start(
                        attns_o[h, qt * 128:(qt + 1) * 128, :], e_tiles[qt][:])

                # ---- eT-path: sT = khT.T-slices @ qhT; exp -> f32r
                et_tiles = []
                for kt2 in range(4):
                    et_t = p_et.tile([128, 2, 512], f32r, name=f"et{h}_{kt2}",
                                     tag="et")
                    et_tiles.append(et_t)
                for kt in range(8):
                    pst = ps_sm.tile([128, 512], f32, name=f"pst{h}_{kt}",
                                     tag="sm")
                    nc.tensor.matmul(pst[:], idb_sb[:], mbT[:, kt, :],
                                     start=True, stop=False)
                    nc.tensor.matmul(
                        pst[:],
                        khT[base:base + 64, hp * 1024 + kt * 128:
                            hp * 1024 + (kt + 1) * 128],
                        qh_sl,
                        start=False, stop=True)
                    nc.scalar.activation(et_tiles[kt // 2][:, kt % 2, :],
                                         pst[:], AF.Exp, scale=INV_TEMPER)

                # ---- row sums of eT (per q) and attn @ V
                poh = ps_oh.tile([128, 512], f32, name=f"poh{h}", tag="oh")
                prt = ps_sm.tile([1, 512], f32, name=f"prt{h}", tag="sm")
                for kt in range(8):
                    nc.tensor.matmul(prt[:], ones_sb[:],
                                     et_tiles[kt // 2][:, kt % 2, :],
                                     start=(kt == 0), stop=(kt == 7))
                for kt in range(8):
                    nc.tensor.matmul(
                        poh[0:64, :],
                        vh[:, kt * 1024 + h * 64:kt * 1024 + (h + 1) * 64],
                        et_tiles[kt // 2][:, kt % 2, :],
                        start=(kt == 0), stop=(kt == 7))
                rt_t = p_rt.tile([1, 512], f32, name=f"rt{h}", tag="rt")
                nc.scalar.copy(rt_t[:], prt[:])
                nc.vector.reciprocal(rt_t[:], rt_t[:])
                rb_t = p_rb.tile([64, 512], f32, name=f"rb{h}", tag="rb")
                nc.gpsimd.partition_broadcast(rb_t[:], rt_t[0:1, :])
                nc.vector.tensor_tensor(
                    ocT[base:base + 64, hp * 512:(hp + 1) * 512],
                    poh[0:64, :], rb_t[:], ALU.mult)

            # =============== PHASE 3: proj + residual + BN stats ===========
            st_l = p_st.tile([128, 16], f32, name="st_l", tag="st")
            x_tiles = []
            for dmt in range(8):
                psx = ps_sm.tile([128, 512], f32, name=f"psx{dmt}", tag="sm")
                for fc in range(8):
                    nc.tensor.matmul(
                        psx[:],
                        pwT_sb[fc][:, dmt * 128:(dmt + 1) * 128],
                        ocT[:, fc * 512:(fc + 1) * 512],
                        start=(fc == 0), stop=(fc == 7))
                x_t = p_x.tile([128, 512], f32, name=f"x{dmt}", tag="x")
                nc.sync.dma_start(
                    x_t[:],
                    qT_d[dmt * 128:(dmt + 1) * 128, :].bitcast(f32))
                nc.vector.tensor_tensor(x_t[:], psx[:], x_t[:], ALU.add)
                x_tiles.append(x_t)
                nc.vector.tensor_reduce(st_l[:, dmt:dmt + 1], x_t[:],
                                        axis=mybir.AxisListType.X, op=ALU.add)
                sq_scr = ps_big.tile([128, 1024], f32, name=f"sq{dmt}",
                                     tag="big")
                nc.scalar.activation(sq_scr[:, 0:512], x_t[:], AF.Square,
                                     accum_out=st_l[:, 8 + dmt:9 + dmt])

            # ---- AllReduce of stats across all 8 cores
            nc.sync.dma_start(st_in_d[:], st_l[:])
            nc.gpsimd.collective_compute(
                "AllReduce", ALU.add,
                ins=[st_in_d[:]],
                outs=[st_out_d[:]],
                replica_groups=[list(range(NCORES))],
            )
            st_g = p_st.tile([128, 16], f32, name="st_g", tag="st2")
            nc.sync.dma_start(st_g[:], st_out_d[:])

            # ---- finalize: mean, var, scale/shift
            fin = p_st.tile([128, 40], f32, name="fin", tag="fin")
            mu_v = fin[:, 0:8]
            ex2_v = fin[:, 8:16]
            var_v = fin[:, 16:24]
            a_v = fin[:, 24:32]
            b_v = fin[:, 32:40]
            nc.vector.tensor_scalar(mu_v, st_g[:, 0:8], 1.0 / NTOK, None,
                                    ALU.mult)
            nc.vector.tensor_scalar(ex2_v, st_g[:, 8:16], 1.0 / NTOK, None,
                                    ALU.mult)
            # var = E[x^2] - mu^2 ; sd = sqrt(var + eps) ; rsd = 1/sd
            eps_t = p_st.tile([128, 1], f32, name="eps_t", tag="eps")
            nc.vector.memset(eps_t[:], BN_EPS)
            nc.scalar.activation(var_v, mu_v, AF.Square)
            nc.vector.tensor_sub(var_v, ex2_v, var_v)
            nc.scalar.activation(var_v, var_v, AF.Sqrt, bias=eps_t[:])
            nc.vector.reciprocal(var_v, var_v)       # var_v now holds rsd
            nc.vector.tensor_tensor(a_v, gb_sb[:, 0:8], var_v, ALU.mult)
            nc.vector.tensor_tensor(b_v, mu_v, a_v, ALU.mult)
            nc.vector.tensor_sub(b_v, gb_sb[:, 8:16], b_v)

            for dmt in range(8):
                nc.vector.tensor_scalar(x_tiles[dmt][:], x_tiles[dmt][:],
                                        a_v[:, dmt:dmt + 1],
                                        b_v[:, dmt:dmt + 1],
                                        ALU.mult, ALU.add)
                nc.sync.dma_start(xT_o[dmt * 128:(dmt + 1) * 128, :],
                                  x_tiles[dmt][:])

    nc.compile()
    return nc


def _prep_shared(w_qs, w_ks, w_vs, proj_w, bn_gamma, bn_beta):
    import ml_dtypes
    wq_p = np.stack([np.concatenate([w_qs[2 * p], w_qs[2 * p + 1]], axis=1)
                     for p in range(8)]).astype(np.float32)
    wk_p = np.stack([np.concatenate([w_ks[2 * p], w_ks[2 * p + 1]], axis=1)
                     for p in range(8)]).astype(np.float32)
    wv_p = np.ascontiguousarray(
        w_vs.transpose(1, 0, 2).reshape(D, N_HEAD * DK)).astype(np.float32)
    pwT_p = np.ascontiguousarray(proj_w.T).astype(np.float32)
    gb_p = np.ascontiguousarray(np.concatenate(
        [bn_gamma.reshape(8, 128).T, bn_beta.reshape(8, 128).T],
        axis=1)).astype(np.float32)
    idb = np.eye(128, dtype=ml_dtypes.bfloat16)
    ones = np.ones((128, 1), dtype=np.float32)
    return dict(wq=wq_p, wk=wk_p, wv=wv_p, pwT=pwT_p, gb=gb_p, idb=idb,
                ones=ones)


def kernel(q, k, v, attn_mask, w_qs, w_ks, w_vs, proj_w, proj_b,
           bn_gamma, bn_beta):
    from concourse.bass_utils import run_bass_kernel_spmd

    q = np.asarray(q, dtype=np.float32)
    k = np.asarray(k, dtype=np.float32)
    v = np.asarray(v, dtype=np.float32)
    attn_mask = np.asarray(attn_mask)

    if "nc" not in _CACHE:
        _CACHE["nc"] = _build_program()
    nc = _CACHE["nc"]

    shared = _prep_shared(np.asarray(w_qs, np.float32),
                          np.asarray(w_ks, np.float32),
                          np.asarray(w_vs, np.float32),
                          np.asarray(proj_w, np.float32),
                          np.asarray(bn_gamma, np.float32),
                          np.asarray(bn_beta, np.float32))

    in_maps = []
    for i in range(NCORES):
        b, th = i // 2, i % 2
        qsl = slice(th * QSH, (th + 1) * QSH)
        m = dict(shared)
        m["qT"] = np.ascontiguousarray(q[b].T[:, qsl])
        m["kT"] = np.ascontiguousarray(k[b].T)
        m["vT"] = np.ascontiguousarray(v[b].T)
        m["msk"] = np.ascontiguousarray(attn_mask[b, qsl, :]).view(np.uint8)
        m["mskT"] = np.ascontiguousarray(attn_mask[b, qsl, :].T).view(np.uint8)
        in_maps.append(m)

    res = run_bass_kernel_spmd(nc, in_maps, core_ids=list(range(NCORES)))

    x = np.empty((B, L, D), dtype=np.float32)
    attns = np.empty((N_HEAD * B, L, L), dtype=np.float32)
    for i in range(NCORES):
        b, th = i // 2, i % 2
        qsl = slice(th * QSH, (th + 1) * QSH)
        r = res.results[i]
        x[b, qsl, :] = r["xT_o"].T
        a = r["attns_o"]
        for h in range(N_HEAD):
            attns[h * B + b, qsl, :] = a[h]
    return x, attns


# revision 12
# speedup vs baseline: 134.4643x; 134.4643x over previous
"""Trainium2 Bass kernel for 16-head attention + proj + residual + BatchNorm.

Shapes (hardcoded): B=4, L=1024, D_MODEL=1024, N_HEAD=16, D_K=D_V=64.
Sharding: 8 cores = (batch b, query-half). Each core computes all 16 heads
for its 512 query tokens; K/V projections are recomputed per query-half
(cheaper than a cross-core reduction of the row-parallel proj matmul).
Only collective: an 8KB AllReduce of BatchNorm statistics.

Layout strategy: all host-side transposes (q/k/v passed d-major, proj_w
passed pre-transposed, masks passed in both orientations) so the device
never runs PE transposes. Scores are computed in both orientations on PE
(s for the attns output + softmax row-sums via ACT accum_out; sT for the
attn@V matmul). The boolean mask is folded into the score PSUM via a bf16
identity-matmul accumulation of (-1e6 * mask), making exp() produce exact
zeros at masked positions.
"""

import sys

if "/opt/trn_rl_repo" not in sys.path:
    sys.path.insert(0, "/opt/trn_rl_repo")

import numpy as np

N_HEAD, D, DK = 16, 1024, 64
B, L = 4, 1024
NCORES = 8
QSH = 512          # query tokens per core
INV_TEMPER = 1.0 / 32.0   # 1/sqrt(D_MODEL)
MASK_NEG = -1.0e6
BN_EPS = 1e-5
NTOK = B * L       # 4096 tokens for batch-norm stats

_CACHE = {}


def _build_program(collective=True, n_iters=1):
    import concourse.bass as bass
    import concourse.tile as tile
    from concourse import bacc, mybir
    from contextlib import ExitStack

    f32 = mybir.dt.float32
    f32r = mybir.dt.float32r
    bf16 = mybir.dt.bfloat16
    u8 = mybir.dt.uint8
    AF = mybir.ActivationFunctionType
    ALU = mybir.AluOpType

    nc = bacc.Bacc("TRN2", target_bir_lowering=False, debug=False,
                   num_devices=NCORES)

    # ---------------- DRAM I/O ----------------
    qT_d = nc.dram_tensor("qT", [D, QSH], f32r, kind="ExternalInput")
    kT_d = nc.dram_tensor("kT", [D, L], f32r, kind="ExternalInput")
    vT_d = nc.dram_tensor("vT", [D, L], f32r, kind="ExternalInput")
    msk_d = nc.dram_tensor("msk", [QSH, L], u8, kind="ExternalInput")
    mskT_d = nc.dram_tensor("mskT", [L, QSH], u8, kind="ExternalInput")
    wq_d = nc.dram_tensor("wq", [8, D, 128], f32r, kind="ExternalInput")
    wk_d = nc.dram_tensor("wk", [8, D, 128], f32r, kind="ExternalInput")
    wv_d = nc.dram_tensor("wv", [D, N_HEAD * DK], f32r, kind="ExternalInput")
    pwT_d = nc.dram_tensor("pwT", [D, D], f32r, kind="ExternalInput")
    gb_d = nc.dram_tensor("gb", [128, 16], f32, kind="ExternalInput")
    idb_d = nc.dram_tensor("idb", [128, 128], bf16, kind="ExternalInput")
    ones_d = nc.dram_tensor("ones", [128, 1], f32r, kind="ExternalInput")

    attns_o = nc.dram_tensor("attns_o", [N_HEAD, QSH, L], f32,
                             kind="ExternalOutput")
    xT_o = nc.dram_tensor("xT_o", [D, QSH], f32, kind="ExternalOutput")

    st_in_d = nc.dram_tensor("st_in", [128, 16], f32)
    st_out_d = nc.dram_tensor("st_out", [128, 16], f32, addr_space="Shared")

    with tile.TileContext(nc) as tc, ExitStack() as glob:
        pool_g = glob.enter_context(tc.tile_pool(name="glob", bufs=1))
        ps_big = glob.enter_context(
            tc.tile_pool(name="psbig", bufs=2, space="PSUM"))
        ps_sm = glob.enter_context(
            tc.tile_pool(name="pssm", bufs=2, space="PSUM"))
        ps_oh = glob.enter_context(
            tc.tile_pool(name="psoh", bufs=2, space="PSUM"))

        # ---- global SBUF residents
        idb_sb = pool_g.tile([128, 128], bf16)
        nc.sync.dma_start(idb_sb[:], idb_d[:])
        ones_sb = pool_g.tile([128, 1], f32r)
        nc.sync.dma_start(ones_sb[:], ones_d[:])
        gb_sb = pool_g.tile([128, 16], f32)
        nc.sync.dma_start(gb_sb[:], gb_d[:])

        import contextlib
        loop_cm = tc.For_i(0, n_iters, 1) if n_iters > 1 else \
            contextlib.nullcontext()
        glob.enter_context(loop_cm)

        qhT = pool_g.tile([128, 8 * 512], f32r)    # [2*64dk, hp*512 + q]
        khT = pool_g.tile([128, 8 * 1024], f32r)   # [2*64dk, hp*1024 + k]
        vh = pool_g.tile([128, 8, 16, 65], f32r)   # [k%128, kt, h, dv|ones]
        ocT = pool_g.tile([128, 8 * 512], f32r)    # [f%128, fc*512 + q]

        # =============== PHASE 1: projections ===============
        with ExitStack() as ph1:
            p_kv = ph1.enter_context(tc.tile_pool(name="kvT", bufs=9))
            p_qt = ph1.enter_context(tc.tile_pool(name="qTs", bufs=8))
            p_w = ph1.enter_context(tc.tile_pool(name="wstr", bufs=4))
            p_wv = ph1.enter_context(tc.tile_pool(name="wvs", bufs=8))

            qT_sb = []
            for c in range(8):
                t = p_qt.tile([128, QSH], f32r, name=f"qT{c}", tag="qT")
                nc.sync.dma_start(t[:], qT_d[c * 128:(c + 1) * 128, :])
                qT_sb.append(t)
            # Q projections first (small DMA footprint -> PE starts early)
            for hp in range(8):
                wq_t = p_w.tile([128, 8, 128], f32r, name=f"wq{hp}", tag="w")
                nc.sync.dma_start(
                    wq_t[:], wq_d[hp].rearrange("(c p) m -> p c m", p=128))
                psq = ps_sm.tile([128, 512], f32, name=f"psq{hp}", tag="sm")
                for c in range(8):
                    nc.tensor.matmul(psq[:], wq_t[:, c, :], qT_sb[c][:],
                                     start=(c == 0), stop=(c == 7))
                nc.scalar.copy(qhT[:, hp * 512:(hp + 1) * 512], psq[:])

            kT_sb = []
            for c in range(8):
                t = p_kv.tile([128, L], f32r, name=f"kT{c}", tag="kvT")
                nc.sync.dma_start(t[:], kT_d[c * 128:(c + 1) * 128, :])
                kT_sb.append(t)

            for hp in range(8):
                wk_t = p_w.tile([128, 8, 128], f32r, name=f"wk{hp}", tag="w")
                nc.sync.dma_start(
                    wk_t[:], wk_d[hp].rearrange("(c p) m -> p c m", p=128))
                psk = ps_big.tile([128, 1024], f32, name=f"psk{hp}", tag="big")
                for half in range(2):
                    sl = slice(half * 512, (half + 1) * 512)
                    for c in range(8):
                        nc.tensor.matmul(psk[:, sl], wk_t[:, c, :],
                                         kT_sb[c][:, sl],
                                         start=(c == 0), stop=(c == 7))
                nc.scalar.copy(khT[:, hp * 1024:(hp + 1) * 1024], psk[:])

            # V projection: vh[kt] = vT[:, kt-chunk].T @ wv  -> (128, 16*64)
            wv_sb = []
            for c in range(8):
                t = p_wv.tile([128, 1024], f32r, name=f"wv{c}", tag="wv")
                nc.sync.dma_start(t[:], wv_d[c * 128:(c + 1) * 128, :])
                wv_sb.append(t)
            vT_sb = []
            for c in range(8):
                t = p_kv.tile([128, L], f32r, name=f"vT{c}", tag="kvT")
                nc.sync.dma_start(t[:], vT_d[c * 128:(c + 1) * 128, :])
                vT_sb.append(t)
            for kt in range(8):
                psv = ps_big.tile([128, 1024], f32, name=f"psv{kt}", tag="big")
                for half in range(2):
                    sl = slice(half * 512, (half + 1) * 512)
                    for c in range(8):
                        nc.tensor.matmul(
                            psv[:, sl],
                            vT_sb[c][:, kt * 128:(kt + 1) * 128],
                            wv_sb[c][:, sl],
                            start=(c == 0), stop=(c == 7))
                nc.scalar.copy(vh[:, kt, :, 0:64],
                               psv[:].rearrange("p (h d) -> p h d", d=64))
                nc.vector.memset(vh[:, kt, :, 64:65].bitcast(f32), 1.0)

        # =============== PHASE 2: attention ===============
        with ExitStack() as ph2:
            p_mu = ph2.enter_context(tc.tile_pool(name="mu8", bufs=1))
            p_mb = ph2.enter_context(tc.tile_pool(name="mbias", bufs=1))
            p_e = ph2.enter_context(tc.tile_pool(name="epool", bufs=4))
            p_et = ph2.enter_context(tc.tile_pool(name="etpool", bufs=4))
            p_r = ph2.enter_context(tc.tile_pool(name="rpool", bufs=1))
            p_rt = ph2.enter_context(tc.tile_pool(name="rtpool", bufs=2))
            p_rb = ph2.enter_context(tc.tile_pool(name="rbpool", bufs=2))
            p_pw = ph2.enter_context(tc.tile_pool(name="pwpool", bufs=8))
            p_x = ph2.enter_context(tc.tile_pool(name="xpool", bufs=9))
            p_st = ph2.enter_context(tc.tile_pool(name="stpool", bufs=1))

            # masks -> additive bias (bf16)
            mu_t = p_mu.tile([128, 4, 1024], u8, name="mu", tag="mu")
            nc.sync.dma_start(mu_t[:], msk_d[:].rearrange("(t p) k -> p t k",
                                                          p=128))
            mb = p_mb.tile([128, 4, 1024], bf16, name="mb", tag="mb")
            nc.vector.tensor_scalar(mb[:], mu_t[:], MASK_NEG, None, ALU.mult)
            muT_t = p_mu.tile([128, 8, 512], u8, name="muT", tag="mu")
            nc.sync.dma_start(muT_t[:], mskT_d[:].rearrange("(t p) k -> p t k",
                                                            p=128))
            mbT = p_mb.tile([128, 8, 512], bf16, name="mbT", tag="mbT")
            nc.vector.tensor_scalar(mbT[:], muT_t[:], MASK_NEG, None, ALU.mult)

            # proj weights loaded here (SBUF freed by phase-1 exit)
            pwT_sb = []
            for c in range(8):
                t = p_pw.tile([128, 1024], f32r, name=f"pwT{c}", tag="pw")
                nc.sync.dma_start(t[:], pwT_d[c * 128:(c + 1) * 128, :])
                pwT_sb.append(t)

            r_all = p_r.tile([128, 64], f32, name="r_all", tag="r")
            ri_all = p_r.tile([128, 64], f32, name="ri_all", tag="ri")

            for h in range(N_HEAD):
                hp, hb = h // 2, h % 2
                base = hb * 64
                qh_sl = qhT[base:base + 64, hp * 512:(hp + 1) * 512]

                # ---- e-path: s = qh @ khT + maskbias; softmax numerator
                e_tiles = []
                for qt in range(4):
                    pse = ps_big.tile([128, 1024], f32, name=f"pse{h}_{qt}",
                                      tag="big")
                    for half in range(2):
                        sl = slice(half * 512, (half + 1) * 512)
                        nc.tensor.matmul(pse[:, sl], idb_sb[:],
                                         mb[:, qt, sl],
                                         start=True, stop=False)
                        nc.tensor.matmul(
                            pse[:, sl],
                            qhT[base:base + 64,
                                hp * 512 + qt * 128:hp * 512 + (qt + 1) * 128],
                            khT[base:base + 64, hp * 1024 + half * 512:
                                hp * 1024 + (half + 1) * 512],
                            start=False, stop=True)
                    e_t = p_e.tile([128, 1024], f32, name=f"e{h}_{qt}",
                                   tag="e")
                    nc.scalar.activation(e_t[:], pse[:], AF.Exp,
                                         scale=INV_TEMPER,
                                         accum_out=r_all[:, h * 4 + qt:h * 4 + qt + 1])
                    e_tiles.append(e_t)
                nc.vector.reciprocal(ri_all[:, h * 4:h * 4 + 4],
                                     r_all[:, h * 4:h * 4 + 4])
                for qt in range(4):
                    nc.vector.tensor_scalar(e_tiles[qt][:], e_tiles[qt][:],
                                            ri_all[:, h * 4 + qt:h * 4 + qt + 1],
                                            None, ALU.mult)
                    nc.sync.dma_start(
                        attns_o[h, qt * 128:(qt + 1) * 128, :], e_tiles[qt][:])

                # ---- eT-path: sT = khT.T-slices @ qhT; exp -> f32r
                et_tiles = []
                for kt2 in range(4):
                    et_t = p_et.tile([128, 2, 512], f32r, name=f"et{h}_{kt2}",
                                     tag="et")
                    et_tiles.append(et_t)
                for kt in range(8):
                    pst = ps_sm.tile([128, 512], f32, name=f"pst{h}_{kt}",
                                     tag="sm")
                    nc.tensor.matmul(pst[:], idb_sb[:], mbT[:, kt, :],
                                     start=True, stop=False)
                    nc.tensor.matmul(
                        pst[:],
                        khT[base:base + 64, hp * 1024 + kt * 128:
                            hp * 1024 + (kt + 1) * 128],
                        qh_sl,
                        start=False, stop=True)
                    nc.scalar.activation(et_tiles[kt // 2][:, kt % 2, :],
                                         pst[:], AF.Exp, scale=INV_TEMPER)

                # ---- row sums of eT (per q) and attn @ V
                poh = ps_oh.tile([128, 512], f32, name=f"poh{h}", tag="oh")
                for kt in range(8):
                    nc.tensor.matmul(
                        poh[0:65, :],
                        vh[:, kt, h, :],
                        et_tiles[kt // 2][:, kt % 2, :],
                        start=(kt == 0), stop=(kt == 7))
                rt_t = p_rt.tile([1, 512], f32, name=f"rt{h}", tag="rt")
                nc.scalar.copy(rt_t[:], poh[64:65, :])
                nc.vector.reciprocal(rt_t[:], rt_t[:])
                rb_t = p_rb.tile([64, 512], f32, name=f"rb{h}", tag="rb")
                nc.gpsimd.partition_broadcast(rb_t[:], rt_t[0:1, :])
                nc.vector.tensor_tensor(
                    ocT[base:base + 64, hp * 512:(hp + 1) * 512],
                    poh[0:64, :], rb_t[:], ALU.mult)

            # =============== PHASE 3: proj + residual + BN stats ===========
            st_l = p_st.tile([128, 16], f32, name="st_l", tag="st")
            x_tiles = []
            for dmt in range(8):
                psx = ps_sm.tile([128, 512], f32, name=f"psx{dmt}", tag="sm")
                for fc in range(8):
                    nc.tensor.matmul(
                        psx[:],
                        pwT_sb[fc][:, dmt * 128:(dmt + 1) * 128],
                        ocT[:, fc * 512:(fc + 1) * 512],
                        start=(fc == 0), stop=(fc == 7))
                x_t = p_x.tile([128, 512], f32, name=f"x{dmt}", tag="x")
                nc.sync.dma_start(
                    x_t[:],
                    qT_d[dmt * 128:(dmt + 1) * 128, :].bitcast(f32))
                nc.vector.tensor_tensor(x_t[:], psx[:], x_t[:], ALU.add)
                x_tiles.append(x_t)
                nc.vector.tensor_reduce(st_l[:, dmt:dmt + 1], x_t[:],
                                        axis=mybir.AxisListType.X, op=ALU.add)
                sq_scr = ps_big.tile([128, 1024], f32, name=f"sq{dmt}",
                                     tag="big")
                nc.scalar.activation(sq_scr[:, 0:512], x_t[:], AF.Square,
                                     accum_out=st_l[:, 8 + dmt:9 + dmt])

            # ---- AllReduce of stats across all 8 cores
            if collective:
                nc.sync.dma_start(st_in_d[:], st_l[:])
                nc.gpsimd.collective_compute(
                    "AllReduce", ALU.add,
                    ins=[st_in_d[:]],
                    outs=[st_out_d[:]],
                    replica_groups=[list(range(NCORES))],
                )
                st_g = p_st.tile([128, 16], f32, name="st_g", tag="st2")
                nc.sync.dma_start(st_g[:], st_out_d[:])
            else:
                st_g = p_st.tile([128, 16], f32, name="st_g", tag="st2")
                nc.vector.tensor_copy(st_g[:], st_l[:])

            # ---- finalize: mean, var, scale/shift
            fin = p_st.tile([128, 40], f32, name="fin", tag="fin")
            mu_v = fin[:, 0:8]
            ex2_v = fin[:, 8:16]
            var_v = fin[:, 16:24]
            a_v = fin[:, 24:32]
            b_v = fin[:, 32:40]
            nc.vector.tensor_scalar(mu_v, st_g[:, 0:8], 1.0 / NTOK, None,
                                    ALU.mult)
            nc.vector.tensor_scalar(ex2_v, st_g[:, 8:16], 1.0 / NTOK, None,
                                    ALU.mult)
            # var = E[x^2] - mu^2 ; sd = sqrt(var + eps) ; rsd = 1/sd
            eps_t = p_st.tile([128, 1], f32, name="eps_t", tag="eps")
            nc.vector.memset(eps_t[:], BN_EPS)
            nc.scalar.activation(var_v, mu_v, AF.Square)
            nc.vector.tensor_sub(var_v, ex2_v, var_v)
            nc.scalar.activation(var_v, var_v, AF.Sqrt, bias=eps_t[:])
            nc.vector.reciprocal(var_v, var_v)       # var_v now holds rsd
            nc.vector.tensor_tensor(a_v, gb_sb[:, 0:8], var_v, ALU.mult)
            nc.vector.tensor_tensor(b_v, mu_v, a_v, ALU.mult)
            nc.vector.tensor_sub(b_v, gb_sb[:, 8:16], b_v)

            for dmt in range(8):
                nc.vector.tensor_scalar(x_tiles[dmt][:], x_tiles[dmt][:],
                                        a_v[:, dmt:dmt + 1],
                                        b_v[:, dmt:dmt + 1],
                                        ALU.mult, ALU.add)
                nc.sync.dma_start(xT_o[dmt * 128:(dmt + 1) * 128, :],
                                  x_tiles[dmt][:])

    nc.compile()
    return nc


def _prep_shared(w_qs, w_ks, w_vs, proj_w, bn_gamma, bn_beta):
    import ml_dtypes
    wq_p = np.stack([np.concatenate([w_qs[2 * p], w_qs[2 * p + 1]], axis=1)
                     for p in range(8)]).astype(np.float32)
    wk_p = np.stack([np.concatenate([w_ks[2 * p], w_ks[2 * p + 1]], axis=1)
                     for p in range(8)]).astype(np.float32)
    wv_p = np.ascontiguousarray(
        w_vs.transpose(1, 0, 2).reshape(D, N_HEAD * DK)).astype(np.float32)
    pwT_p = np.ascontiguousarray(proj_w.T).astype(np.float32)
    gb_p = np.ascontiguousarray(np.concatenate(
        [bn_gamma.reshape(8, 128).T, bn_beta.reshape(8, 128).T],
        axis=1)).astype(np.float32)
    idb = np.eye(128, dtype=ml_dtypes.bfloat16)
    ones = np.ones((128, 1), dtype=np.float32)
    return dict(wq=wq_p, wk=wk_p, wv=wv_p, pwT=pwT_p, gb=gb_p, idb=idb,
                ones=ones)


def kernel(q, k, v, attn_mask, w_qs, w_ks, w_vs, proj_w, proj_b,
           bn_gamma, bn_beta):
    from concourse.bass_utils import run_bass_kernel_spmd

    q = np.asarray(q, dtype=np.float32)
    k = np.asarray(k, dtype=np.float32)
    v = np.asarray(v, dtype=np.float32)
    attn_mask = np.asarray(attn_mask)

    if "nc" not in _CACHE:
        _CACHE["nc"] = _build_program()
    nc = _CACHE["nc"]

    shared = _prep_shared(np.asarray(w_qs, np.float32),
                          np.asarray(w_ks, np.float32),
                          np.asarray(w_vs, np.float32),
                          np.asarray(proj_w, np.float32),
                          np.asarray(bn_gamma, np.float32),
                          np.asarray(bn_beta, np.float32))

    in_maps = []
    for i in range(NCORES):
        b, th = i // 2, i % 2
        qsl = slice(th * QSH, (th + 1) * QSH)
        m = dict(shared)
        m["qT"] = np.ascontiguousarray(q[b].T[:, qsl])
        m["kT"] = np.ascontiguousarray(k[b].T)
        m["vT"] = np.ascontiguousarray(v[b].T)
        m["msk"] = np.ascontiguousarray(attn_mask[b, qsl, :]).view(np.uint8)
        m["mskT"] = np.ascontiguousarray(attn_mask[b, qsl, :].T).view(np.uint8)
        in_maps.append(m)

    res = run_bass_kernel_spmd(nc, in_maps, core_ids=list(range(NCORES)))

    x = np.empty((B, L, D), dtype=np.float32)
    attns = np.empty((N_HEAD * B, L, L), dtype=np.float32)
    for i in range(NCORES):
        b, th = i // 2, i % 2
        qsl = slice(th * QSH, (th + 1) * QSH)
        r = res.results[i]
        x[b, qsl, :] = r["xT_o"].T
        a = r["attns_o"]
        for h in range(N_HEAD):
            attns[h * B + b, qsl, :] = a[h]
    return x, attns


# revision 16
# speedup vs baseline: 280.8642x; 2.0888x over previous
"""Trainium2 Bass kernel for 16-head attention + proj + residual + BatchNorm.

Shapes (hardcoded): B=4, L=1024, D_MODEL=1024, N_HEAD=16, D_K=D_V=64.
Sharding: 8 cores = (batch b, query-half). Each core computes all 16 heads
for its 512 query tokens; K/V projections are recomputed per query-half
(cheaper than a cross-core reduction of the row-parallel proj matmul).
Only collective: an 8KB AllReduce of BatchNorm statistics.

Layout strategy: all host-side transposes (q/k/v passed d-major, proj_w
passed pre-transposed, masks passed in both orientations) so the device
never runs PE transposes. Scores are computed in both orientations on PE
(s for the attns output + softmax row-sums via ACT accum_out; sT for the
attn@V matmul). The boolean mask is folded into the score PSUM via a bf16
identity-matmul accumulation of (-1e6 * mask), making exp() produce exact
zeros at masked positions.
"""

import sys

if "/opt/trn_rl_repo" not in sys.path:
    sys.path.insert(0, "/opt/trn_rl_repo")

import numpy as np

N_HEAD, D, DK = 16, 1024, 64
B, L = 4, 1024
NCORES = 8
QSH = 512          # query tokens per core
INV_TEMPER = 1.0 / 32.0   # 1/sqrt(D_MODEL)
MASK_NEG = -1.0e6
BN_EPS = 1e-5
NTOK = B * L       # 4096 tokens for batch-norm stats

_CACHE = {}


def _build_program(collective=True, n_iters=1):
    import concourse.bass as bass
    import concourse.tile as tile
    from concourse import bacc, mybir
    from contextlib import ExitStack

    f32 = mybir.dt.float32
    f32r = mybir.dt.float32r
    bf16 = mybir.dt.bfloat16
    u8 = mybir.dt.uint8
    AF = mybir.ActivationFunctionType
    ALU = mybir.AluOpType

    nc = bacc.Bacc("TRN2", target_bir_lowering=False, debug=False,
                   num_devices=NCORES)

    # ---------------- DRAM I/O ----------------
    qT_d = nc.dram_tensor("qT", [D, QSH], f32r, kind="ExternalInput")
    kT_d = nc.dram_tensor("kT", [D, L], f32r, kind="ExternalInput")
    vT_d = nc.dram_tensor("vT", [D, L], f32r, kind="ExternalInput")
    mskT_d = nc.dram_tensor("mskT", [L, QSH], u8, kind="ExternalInput")
    wq_d = nc.dram_tensor("wq", [8, D, 128], f32r, kind="ExternalInput")
    wk_d = nc.dram_tensor("wk", [8, D, 128], f32r, kind="ExternalInput")
    wv_d = nc.dram_tensor("wv", [D, N_HEAD * DK], f32r, kind="ExternalInput")
    pwT_d = nc.dram_tensor("pwT", [D, D], f32r, kind="ExternalInput")
    gb_d = nc.dram_tensor("gb", [128, 16], f32, kind="ExternalInput")
    idb_d = nc.dram_tensor("idb", [128, 128], bf16, kind="ExternalInput")
    ones_d = nc.dram_tensor("ones", [128, 1], f32r, kind="ExternalInput")

    attns_o = nc.dram_tensor("attns_o", [N_HEAD, L, QSH], f32,
                             kind="ExternalOutput")
    xT_o = nc.dram_tensor("xT_o", [D, QSH], f32, kind="ExternalOutput")

    st_in_d = nc.dram_tensor("st_in", [128, 16], f32)
    st_out_d = nc.dram_tensor("st_out", [128, 16], f32, addr_space="Shared")

    with tile.TileContext(nc) as tc, ExitStack() as glob:
        pool_g = glob.enter_context(tc.tile_pool(name="glob", bufs=1))
        ps_big = glob.enter_context(
            tc.tile_pool(name="psbig", bufs=2, space="PSUM"))
        ps_sm = glob.enter_context(
            tc.tile_pool(name="pssm", bufs=2, space="PSUM"))
        ps_oh = glob.enter_context(
            tc.tile_pool(name="psoh", bufs=2, space="PSUM"))

        # ---- global SBUF residents
        idb_sb = pool_g.tile([128, 128], bf16)
        nc.sync.dma_start(idb_sb[:], idb_d[:])
        ones_sb = pool_g.tile([128, 1], f32r)
        nc.sync.dma_start(ones_sb[:], ones_d[:])
        gb_sb = pool_g.tile([128, 16], f32)
        nc.sync.dma_start(gb_sb[:], gb_d[:])

        import contextlib
        loop_cm = tc.For_i(0, n_iters, 1) if n_iters > 1 else \
            contextlib.nullcontext()
        glob.enter_context(loop_cm)

        qhT = pool_g.tile([128, 8 * 512], f32r)    # [2*64dk, hp*512 + q]
        khT = pool_g.tile([128, 8 * 1024], f32r)   # [2*64dk, hp*1024 + k]
        vh = pool_g.tile([128, 8, 16, 65], f32r)   # [k%128, kt, h, dv|ones]
        ocT = pool_g.tile([128, 8 * 512], f32r)    # [f%128, fc*512 + q]

        # =============== PHASE 1: projections ===============
        with ExitStack() as ph1:
            p_kv = ph1.enter_context(tc.tile_pool(name="kvT", bufs=9))
            p_qt = ph1.enter_context(tc.tile_pool(name="qTs", bufs=8))
            p_w = ph1.enter_context(tc.tile_pool(name="wstr", bufs=4))
            p_wv = ph1.enter_context(tc.tile_pool(name="wvs", bufs=8))

            qT_sb = []
            for c in range(8):
                t = p_qt.tile([128, QSH], f32r, name=f"qT{c}", tag="qT")
                nc.sync.dma_start(t[:], qT_d[c * 128:(c + 1) * 128, :])
                qT_sb.append(t)
            # Q projections first (small DMA footprint -> PE starts early)
            for hp in range(8):
                wq_t = p_w.tile([128, 8, 128], f32r, name=f"wq{hp}", tag="w")
                nc.sync.dma_start(
                    wq_t[:], wq_d[hp].rearrange("(c p) m -> p c m", p=128))
                psq = ps_sm.tile([128, 512], f32, name=f"psq{hp}", tag="sm")
                for c in range(8):
                    nc.tensor.matmul(psq[:], wq_t[:, c, :], qT_sb[c][:],
                                     start=(c == 0), stop=(c == 7))
                nc.scalar.copy(qhT[:, hp * 512:(hp + 1) * 512], psq[:])

            kT_sb = []
            for c in range(8):
                t = p_kv.tile([128, L], f32r, name=f"kT{c}", tag="kvT")
                nc.sync.dma_start(t[:], kT_d[c * 128:(c + 1) * 128, :])
                kT_sb.append(t)

            for hp in range(8):
                wk_t = p_w.tile([128, 8, 128], f32r, name=f"wk{hp}", tag="w")
                nc.sync.dma_start(
                    wk_t[:], wk_d[hp].rearrange("(c p) m -> p c m", p=128))
                psk = ps_big.tile([128, 1024], f32, name=f"psk{hp}", tag="big")
                for half in range(2):
                    sl = slice(half * 512, (half + 1) * 512)
                    for c in range(8):
                        nc.tensor.matmul(psk[:, sl], wk_t[:, c, :],
                                         kT_sb[c][:, sl],
                                         start=(c == 0), stop=(c == 7))
                nc.scalar.copy(khT[:, hp * 1024:(hp + 1) * 1024], psk[:])

            # V projection: vh[kt] = vT[:, kt-chunk].T @ wv  -> (128, 16*64)
            wv_sb = []
            for c in range(8):
                t = p_wv.tile([128, 1024], f32r, name=f"wv{c}", tag="wv")
                nc.sync.dma_start(t[:], wv_d[c * 128:(c + 1) * 128, :])
                wv_sb.append(t)
            vT_sb = []
            for c in range(8):
                t = p_kv.tile([128, L], f32r, name=f"vT{c}", tag="kvT")
                nc.sync.dma_start(t[:], vT_d[c * 128:(c + 1) * 128, :])
                vT_sb.append(t)
            for kt in range(8):
                psv = ps_big.tile([128, 1024], f32, name=f"psv{kt}", tag="big")
                for half in range(2):
                    sl = slice(half * 512, (half + 1) * 512)
                    for c in range(8):
                        nc.tensor.matmul(
                            psv[:, sl],
                            vT_sb[c][:, kt * 128:(kt + 1) * 128],
                            wv_sb[c][:, sl],
                            start=(c == 0), stop=(c == 7))
                nc.scalar.copy(vh[:, kt, :, 0:64],
                               psv[:].rearrange("p (h d) -> p h d", d=64))
                nc.vector.memset(vh[:, kt, :, 64:65].bitcast(f32), 1.0)

        # =============== PHASE 2: attention ===============
        with ExitStack() as ph2:
            p_mu = ph2.enter_context(tc.tile_pool(name="mu8", bufs=1))
            p_mb = ph2.enter_context(tc.tile_pool(name="mbias", bufs=1))
            p_et = ph2.enter_context(tc.tile_pool(name="etpool", bufs=8))
            p_rt = ph2.enter_context(tc.tile_pool(name="rtpool", bufs=1))
            p_rb = ph2.enter_context(tc.tile_pool(name="rbpool", bufs=2))
            p_at = ph2.enter_context(tc.tile_pool(name="atpool", bufs=3))
            p_pw = ph2.enter_context(tc.tile_pool(name="pwpool", bufs=8))
            p_x = ph2.enter_context(tc.tile_pool(name="xpool", bufs=8))
            p_st = ph2.enter_context(tc.tile_pool(name="stpool", bufs=1))

            # mask -> additive bias (bf16), transposed orientation only
            muT_t = p_mu.tile([128, 8, 512], u8, name="muT", tag="mu")
            nc.sync.dma_start(muT_t[:], mskT_d[:].rearrange("(t p) k -> p t k",
                                                            p=128))
            mbT = p_mb.tile([128, 8, 512], bf16, name="mbT", tag="mbT")
            nc.vector.tensor_scalar(mbT[:], muT_t[:], MASK_NEG, None, ALU.mult)

            # proj weights loaded here (SBUF freed by phase-1 exit)
            pwT_sb = []
            for c in range(8):
                t = p_pw.tile([128, 1024], f32r, name=f"pwT{c}", tag="pw")
                nc.sync.dma_start(t[:], pwT_d[c * 128:(c + 1) * 128, :])
                pwT_sb.append(t)

            for h in range(N_HEAD):
                hp, hb = h // 2, h % 2
                base = hb * 64
                qh_sl = qhT[base:base + 64, hp * 512:(hp + 1) * 512]

                # ---- sT = khT.T-slices @ qhT + maskT bias; exp -> f32r
                et_tiles = []
                for kt2 in range(4):
                    et_t = p_et.tile([128, 2, 512], f32r, name=f"et{h}_{kt2}",
                                     tag="et")
                    et_tiles.append(et_t)
                for kt in range(8):
                    pst = ps_sm.tile([128, 512], f32, name=f"pst{h}_{kt}",
                                     tag="sm")
                    nc.tensor.matmul(pst[:], idb_sb[:], mbT[:, kt, :],
                                     start=True, stop=False)
                    nc.tensor.matmul(
                        pst[:],
                        khT[base:base + 64, hp * 1024 + kt * 128:
                            hp * 1024 + (kt + 1) * 128],
                        qh_sl,
                        start=False, stop=True)
                    nc.scalar.activation(et_tiles[kt // 2][:, kt % 2, :],
                                         pst[:], AF.Exp, scale=INV_TEMPER)

                # ---- attn @ V with a ones-row: rows 0-63 = out, row 64 = sums
                poh = ps_oh.tile([128, 512], f32, name=f"poh{h}", tag="oh")
                for kt in range(8):
                    nc.tensor.matmul(
                        poh[0:65, :],
                        vh[:, kt, h, :],
                        et_tiles[kt // 2][:, kt % 2, :],
                        start=(kt == 0), stop=(kt == 7))
                rt_t = p_rt.tile([1, 512], f32, name=f"rt{h}", tag="rt")
                nc.scalar.copy(rt_t[:], poh[64:65, :])
                nc.vector.reciprocal(rt_t[:], rt_t[:])
                rb_t = p_rb.tile([128, 512], f32, name=f"rb{h}", tag="rb")
                nc.gpsimd.partition_broadcast(rb_t[:], rt_t[0:1, :])
                nc.vector.tensor_tensor(
                    ocT[base:base + 64, hp * 512:(hp + 1) * 512],
                    poh[0:64, :], rb_t[0:64, :], ALU.mult)

                # ---- normalize eT in place (f32 view) and write attnsT out
                for kt2 in range(4):
                    ev = et_tiles[kt2][:].bitcast(f32)
                    at_t = p_at.tile([128, 2, 512], f32, name=f"at{h}_{kt2}",
                                     tag="at")
                    for sub in range(2):
                        eng = nc.vector if (kt2 * 2 + sub) % 8 < 5 else \
                            nc.gpsimd
                        eng.tensor_tensor(at_t[:, sub, :], ev[:, sub, :],
                                          rb_t[:], ALU.mult)
                    nc.sync.dma_start(
                        attns_o[h, kt2 * 256:(kt2 + 1) * 256, :].rearrange(
                            "(s p) q -> p s q", p=128),
                        at_t[:])

            # =============== PHASE 3: proj + residual + BN stats ===========
            st_l = p_st.tile([128, 16], f32, name="st_l", tag="st")
            x_tiles = []
            for dmt in range(8):
                psx = ps_sm.tile([128, 512], f32, name=f"psx{dmt}", tag="sm")
                for fc in range(8):
                    nc.tensor.matmul(
                        psx[:],
                        pwT_sb[fc][:, dmt * 128:(dmt + 1) * 128],
                        ocT[:, fc * 512:(fc + 1) * 512],
                        start=(fc == 0), stop=(fc == 7))
                x_t = p_x.tile([128, 512], f32, name=f"x{dmt}", tag="x")
                nc.sync.dma_start(
                    x_t[:],
                    qT_d[dmt * 128:(dmt + 1) * 128, :].bitcast(f32))
                nc.vector.tensor_tensor(x_t[:], psx[:], x_t[:], ALU.add)
                x_tiles.append(x_t)
                nc.vector.tensor_reduce(st_l[:, dmt:dmt + 1], x_t[:],
                                        axis=mybir.AxisListType.X, op=ALU.add)
                sq_scr = ps_big.tile([128, 1024], f32, name=f"sq{dmt}",
                                     tag="big")
                nc.scalar.activation(sq_scr[:, 0:512], x_t[:], AF.Square,
                                     accum_out=st_l[:, 8 + dmt:9 + dmt])

            # ---- AllReduce of stats across all 8 cores
            if collective:
                nc.sync.dma_start(st_in_d[:], st_l[:])
                nc.gpsimd.collective_compute(
                    "AllReduce", ALU.add,
                    ins=[st_in_d[:]],
                    outs=[st_out_d[:]],
                    replica_groups=[list(range(NCORES))],
                )
                st_g = p_st.tile([128, 16], f32, name="st_g", tag="st2")
                nc.sync.dma_start(st_g[:], st_out_d[:])
            else:
                st_g = p_st.tile([128, 16], f32, name="st_g", tag="st2")
                nc.vector.tensor_copy(st_g[:], st_l[:])

            # ---- finalize: mean, var, scale/shift
            fin = p_st.tile([128, 40], f32, name="fin", tag="fin")
            mu_v = fin[:, 0:8]
            ex2_v = fin[:, 8:16]
            var_v = fin[:, 16:24]
            a_v = fin[:, 24:32]
            b_v = fin[:, 32:40]
            nc.vector.tensor_scalar(mu_v, st_g[:, 0:8], 1.0 / NTOK, None,
                                    ALU.mult)
            nc.vector.tensor_scalar(ex2_v, st_g[:, 8:16], 1.0 / NTOK, None,
                                    ALU.mult)
            # var = E[x^2] - mu^2 ; sd = sqrt(var + eps) ; rsd = 1/sd
            eps_t = p_st.tile([128, 1], f32, name="eps_t", tag="eps")
            nc.vector.memset(eps_t[:], BN_EPS)
            nc.scalar.activation(var_v, mu_v, AF.Square)
            nc.vector.tensor_sub(var_v, ex2_v, var_v)
            nc.scalar.activation(var_v, var_v, AF.Sqrt, bias=eps_t[:])
            nc.vector.reciprocal(var_v, var_v)       # var_v now holds rsd
            nc.vector.tensor_tensor(a_v, gb_sb[:, 0:8], var_v, ALU.mult)
            nc.vector.tensor_tensor(b_v, mu_v, a_v, ALU.mult)
            nc.vector.tensor_sub(b_v, gb_sb[:, 8:16], b_v)

            for dmt in range(8):
                nc.vector.tensor_scalar(x_tiles[dmt][:], x_tiles[dmt][:],
                                        a_v[:, dmt:dmt + 1],
                                        b_v[:, dmt:dmt + 1],
                                        ALU.mult, ALU.add)
                nc.sync.dma_start(xT_o[dmt * 128:(dmt + 1) * 128, :],
                                  x_tiles[dmt][:])

    nc.compile()
    return nc


def _prep_shared(w_qs, w_ks, w_vs, proj_w, bn_gamma, bn_beta):
    import ml_dtypes
    wq_p = np.stack([np.concatenate([w_qs[2 * p], w_qs[2 * p + 1]], axis=1)
                     for p in range(8)]).astype(np.float32)
    wk_p = np.stack([np.concatenate([w_ks[2 * p], w_ks[2 * p + 1]], axis=1)
                     for p in range(8)]).astype(np.float32)
    wv_p = np.ascontiguousarray(
        w_vs.transpose(1, 0, 2).reshape(D, N_HEAD * DK)).astype(np.float32)
    pwT_p = np.ascontiguousarray(proj_w.T).astype(np.float32)
    gb_p = np.ascontiguousarray(np.concatenate(
        [bn_gamma.reshape(8, 128).T, bn_beta.reshape(8, 128).T],
        axis=1)).astype(np.float32)
    idb = np.eye(128, dtype=ml_dtypes.bfloat16)
    ones = np.ones((128, 1), dtype=np.float32)
    return dict(wq=wq_p, wk=wk_p, wv=wv_p, pwT=pwT_p, gb=gb_p, idb=idb,
                ones=ones)


def kernel(q, k, v, attn_mask, w_qs, w_ks, w_vs, proj_w, proj_b,
           bn_gamma, bn_beta):
    from concourse.bass_utils import run_bass_kernel_spmd

    q = np.asarray(q, dtype=np.float32)
    k = np.asarray(k, dtype=np.float32)
    v = np.asarray(v, dtype=np.float32)
    attn_mask = np.asarray(attn_mask)

    if "nc" not in _CACHE:
        _CACHE["nc"] = _build_program()
    nc = _CACHE["nc"]

    shared = _prep_shared(np.asarray(w_qs, np.float32),
                          np.asarray(w_ks, np.float32),
                          np.asarray(w_vs, np.float32),
                          np.asarray(proj_w, np.float32),
                          np.asarray(bn_gamma, np.float32),
                          np.asarray(bn_beta, np.float32))

    in_maps = []
    for i in range(NCORES):
        b, th = i // 2, i % 2
        qsl = slice(th * QSH, (th + 1) * QSH)
        m = dict(shared)
        m["qT"] = np.ascontiguousarray(q[b].T[:, qsl])
        m["kT"] = np.ascontiguousarray(k[b].T)
        m["vT"] = np.ascontiguousarray(v[b].T)
        m["mskT"] = np.ascontiguousarray(attn_mask[b, qsl, :].T).view(np.uint8)
        in_maps.append(m)

    res = run_bass_kernel_spmd(nc, in_maps, core_ids=list(range(NCORES)))

    x = np.empty((B, L, D), dtype=np.float32)
    attns = np.empty((N_HEAD * B, L, L), dtype=np.float32)
    for i in range(NCORES):
        b, th = i // 2, i % 2
        qsl = slice(th * QSH, (th + 1) * QSH)
        r = res.results[i]
        x[b, qsl, :] = r["xT_o"].T
        a = r["attns_o"]
        for h in range(N_HEAD):
            attns[h * B + b, qsl, :] = a[h].T
    return x, attns
